# revision 12
# baseline (speedup 1.0000x reference)
"""Trainium2 Bass kernel for nn_Attention_4243427688485.

Computation (per batch b):
    a   = z_b @ M @ e_b^T            [N, ME]
    A   = softmax(sigmoid(a), dim=N) (softmax over the query axis N)
    eo  = A @ e_b                    [N, D]
Returns (eo, A) stacked over the batch.

Sharding: data-parallel over batch B=8 across the 8 NeuronCores (one batch
per core, M replicated).  No collectives.  Host uploads fp16 shards
(z/e pre-transposed); outputs come back fp16 and transposed where noted.

Per-core device program:
  - mm1 (fp16): zMT[e',n] = sum_d M[d,e'] z[n,d]
  - mm2 (fp16): aT[m,n]   = sum_e' e[m,e'] zM[n,e']; ScalarE evicts via
    tanh(a/2); softmax over n: t = exp(0.5u+0.5), accum_out row-sum S,
    DVE reciprocal r=1/S, aT16 = t*r (fp16) = the A output (transposed).
  - mm3 (fp8 DoubleRow, ~1.5-2x TensorE rate): exploits the sigmoid
    saturation structure.  t = exp(sigmoid(a)) clusters at exactly {1, e}
    (98.7% of entries saturate), so with the affine split
        t = c + beta*h,  c=(1+e)/2, beta=(e-1)/2,  h in {-1,+1} (mostly)
    h is EXACTLY representable in fp8e4 at the clusters.  Then
        eo[n,d] = c * colsum[d] + beta * sum_m h[m,n] * (e[m,d]/S[m])
    The beta-term runs as fp8e4 DoubleRow matmuls in the eoT orientation:
        eoT = (etil8)^T-style matmul: lhsT = etil8[m,d] = fp8(e*r*2048),
        rhs = h8[m,n], psum accumulates m in 4 double-row (256-wide) steps.
    The c*colsum term is rank-1 in n: the device outputs S ([128,8] fp32,
    4KB) and the host folds  eo += c * ((1/S) @ e)  exactly in fp32 during
    the gather (1M MACs/batch on host, negligible).
    fp8 quantization error on etil dominates: measured rel_err(eo) ~1.1e-2
    vs the 2e-2 gate (A output unchanged at ~2.8e-3).
"""

import numpy as np

import concourse.bass as bass
import concourse.mybir as mybir
import concourse.tile as tile
from concourse import bacc
from concourse.bass_utils import run_bass_kernel_spmd

AF = mybir.ActivationFunctionType
ALU = mybir.AluOpType
DR = mybir.MatmulPerfMode.DoubleRow
F32 = mybir.dt.float32
FP16 = mybir.dt.float16
FP8 = mybir.dt.float8e4

P = 128
NT = 8
SZ = 1024
NC = 8

C_AFF = (1.0 + float(np.e)) / 2.0     # 1.8591409142295225
B_AFF = (float(np.e) - 1.0) / 2.0     # 0.8591409142295225
KQ = 2048.0                           # etil prescale so fp8e4 sees ~unit values


def _build_nc(unroll: int = 1, tiny_io: bool = False, variant: str = "v2") -> bass.Bass:
    nc = bacc.Bacc()

    if tiny_io:
        nc.declare_dram_parameter("tin", [1, 1], F32, isOutput=False)
        dout = nc.declare_dram_parameter("tout", [1, 1], F32, isOutput=True)
        zt_d = nc.dram_tensor("zti", [SZ, SZ], FP16)
        e_d = nc.dram_tensor("ei", [SZ, SZ], FP16)
        et_d = nc.dram_tensor("eti", [SZ, SZ], FP16)
        m_d = nc.dram_tensor("Mi", [SZ, SZ], FP16)
        eo_d = nc.dram_tensor("eoi", [SZ, SZ], FP16)
        a_d = nc.dram_tensor("Ai", [SZ, SZ], FP16)
        s_d = nc.dram_tensor("Si", [P, NT], F32)
    else:
        zt_d = nc.declare_dram_parameter("zT", [SZ, SZ], FP16, isOutput=False)
        e_d = nc.declare_dram_parameter("e", [SZ, SZ], FP16, isOutput=False)
        et_d = nc.declare_dram_parameter("eT", [SZ, SZ], FP16, isOutput=False)
        m_d = nc.declare_dram_parameter("M", [SZ, SZ], FP16, isOutput=False)
        eo_d = nc.declare_dram_parameter("eoT", [SZ, SZ], FP16, isOutput=True)
        a_d = nc.declare_dram_parameter("A", [SZ, SZ], FP16, isOutput=True)
        s_d = nc.declare_dram_parameter("S", [P, NT], F32, isOutput=True)

    ztr = zt_d.rearrange("(j p) d -> j p d", p=P)
    er = e_d.rearrange("(j p) d -> j p d", p=P)
    etr = et_d.rearrange("(j p) d -> j p d", p=P)
    mr = m_d.rearrange("(j p) d -> j p d", p=P)
    eor = eo_d.rearrange("(j p) d -> j p d", p=P)
    ar = a_d.rearrange("(j p) d -> j p d", p=P)

    with tile.TileContext(nc) as tc:
        with (
            tc.tile_pool(name="big", bufs=1) as big,
            tc.tile_pool(name="consts", bufs=1) as consts,
            tc.tile_pool(name="tpool", bufs=4) as tpool,
            tc.tile_pool(name="stage", bufs=8) as stage,
            tc.tile_pool(name="psum_mm", bufs=1, space="PSUM") as pmm,
        ):
            halfb = consts.tile([P, 1], F32)
            nc.any.memset(halfb, 0.5)
            zerob = consts.tile([P, 1], F32)
            nc.any.memset(zerob, 0.0)
            S = consts.tile([P, NT], F32)
            S0 = consts.tile([P, NT], F32)
            S1 = consts.tile([P, NT], F32)
            r = consts.tile([P, NT], F32)

            # v2: zT16/m16 double-buffered (allocated per body in the body
            # emitter); v1 allocates them here as singles.
            if variant == "v1":
                zT16s = big.tile([P, NT, SZ], FP16, name="zT16s")
                m16s = big.tile([P, NT, SZ], FP16, name="m16s")
            else:
                zT16s = m16s = None
            e16 = big.tile([P, NT, SZ], FP16)    # e16[p, jm, d]  = e[jm*128+p, d]
            eT16 = big.tile([P, NT, SZ], FP16)   # eT16[p, je, m] = e[m, je*128+p]
            zMT = big.tile([P, NT, SZ], FP16)    # zMT[p, je, n]  = (z@M)[n, je*128+p]
            u16 = big.tile([P, NT, SZ], FP16)    # u[p, jm, n]    = tanh(a[n, jm*128+p]/2)
            aT16 = big.tile([P, NT, SZ], FP16)   # aT16[p, jm, n] = A[n, jm*128+p]
            # h8/etil8 as 4 per-pair tiles (matching DR jk pairs) so mm3's
            # early matmuls don't wait on the last m-tile's writes (tile
            # dependency tracking is per-tile, not per-slice).
            if variant == "v1":
                h8 = big.tile([P, NT, SZ], FP8, name="h8")
                etil8 = big.tile([P, NT, SZ], FP8, name="etil8")
            else:
                h8 = [big.tile([P, 2, SZ], FP8, name=f"h8_{i}") for i in range(4)]
                etil8 = [big.tile([P, 2, SZ], FP8, name=f"etil8_{i}") for i in range(4)]

            emit = _emit_body if variant == "v2" else _emit_body_v1
            for _ in range(unroll):
                emit(
                    nc, big, pmm, tpool, stage,
                    ztr, er, etr, mr, eor, ar, s_d,
                    e16, eT16, zMT, u16, aT16, h8, etil8,
                    halfb, zerob, S, S0, S1, r, zT16s, m16s,
                )

            if tiny_io:
                dstage = consts.tile([1, 1], F32)
                nc.any.memset(dstage, 1.0)
                nc.sync.dma_start(out=dout[:], in_=dstage[:])

    nc.compile()
    return nc


def _emit_body(nc, big, pmm, tpool, stage, ztr, er, etr, mr, eor, ar, s_d,
               e16, eT16, zMT, u16, aT16, h8, etil8,
               halfb, zerob, S, S0, S1, r, zT16s=None, m16s=None):
    zT16 = big.tile([P, NT, SZ], FP16, tag="zT16", bufs=2, name="zT16")
    m16 = big.tile([P, NT, SZ], FP16, tag="m16", bufs=2, name="m16")
    # ---- loads, in consumption order; zT on the SP DMA queue and m16 on
    # the Activation DMA queue so the mm1 head is fed by two queues ----
    for j in range(NT):
        nc.sync.dma_start(out=zT16[:, j, :], in_=ztr[j])
        nc.scalar.dma_start(out=m16[:, j, :], in_=mr[j])
    for j in range(NT):
        nc.sync.dma_start(out=eT16[:, j, :], in_=etr[j])
    for j in range(NT):
        nc.sync.dma_start(out=e16[:, j, :], in_=er[j])

    # ---- mm1: zMT[e', n] = sum_d M[d, e'] * z[n, d] ----
    for h in range(2):
        for je in range(NT):
            ps = pmm.tile([P, 512], F32, tag="mm", bufs=4)
            for jd in range(NT):
                nc.tensor.matmul(
                    ps[:],
                    m16[:, jd, je * P:(je + 1) * P],
                    zT16[:, jd, h * 512:(h + 1) * 512],
                    start=(jd == 0),
                    stop=(jd == NT - 1),
                )
            nc.scalar.copy(out=zMT[:, je, h * 512:(h + 1) * 512], in_=ps[:])

    # ---- mm2 + fused softmax(sigmoid) per m-tile ----
    for jm in range(NT):
        for h in range(2):
            ps = pmm.tile([P, 512], F32, tag="mm", bufs=4)
            for je in range(NT):
                nc.tensor.matmul(
                    ps[:],
                    eT16[:, je, jm * P:(jm + 1) * P],
                    zMT[:, je, h * 512:(h + 1) * 512],
                    start=(je == 0),
                    stop=(je == NT - 1),
                )
            nc.scalar.activation(
                u16[:, jm, h * 512:(h + 1) * 512], ps[:], AF.Tanh,
                bias=zerob[:], scale=0.5,
            )
        # exp + h8 in n-halves so DVE pipelines behind ScalarE, shortening
        # the jm=7 critical chain into mm3's jk=3 operands.
        t = tpool.tile([P, SZ], FP16, tag="t")
        for hh, Sh in ((0, S0), (1, S1)):
            nc.scalar.activation(
                t[:, hh * 512:(hh + 1) * 512],
                u16[:, jm, hh * 512:(hh + 1) * 512], AF.Exp,
                bias=halfb[:], scale=0.5,
                accum_out=Sh[:, jm:jm + 1],
            )
            nc.vector.tensor_scalar(
                h8[jm // 2][:, jm % 2, hh * 512:(hh + 1) * 512],
                t[:, hh * 512:(hh + 1) * 512], C_AFF, 1.0 / B_AFF,
                op0=ALU.subtract, op1=ALU.mult,
            )
        nc.vector.tensor_scalar_add(S[:, jm:jm + 1], S0[:, jm:jm + 1], S1[:, jm:jm + 1])
        nc.vector.reciprocal(r[:, jm:jm + 1], S[:, jm:jm + 1])
        nc.vector.tensor_scalar(
            etil8[jm // 2][:, jm % 2, :], e16[:, jm, :], r[:, jm:jm + 1], KQ,
            op0=ALU.mult, op1=ALU.mult,
        )
        nc.vector.tensor_scalar_mul(aT16[:, jm, :], t[:], r[:, jm:jm + 1])
        # A output: the fp16 softmax tile goes out directly (stored
        # transposed; host fixes layout and upcasts)
        nc.scalar.dma_start(out=ar[jm], in_=aT16[:, jm, :])

    # S out (host folds the rank-1 c*colsum term during the gather)
    nc.scalar.dma_start(out=s_d[:], in_=S[:])

    # ---- mm3 (fp8 DoubleRow): eoT[d, n] = beta/KQ * sum_m etil8*h8 ----
    # The first four psum tiles are staged (jk=0..2 for all four, then the
    # jk=3 round) so the jk=3 operands — h8/etil8 of the last two m-tiles,
    # which are only ready ~2us after mm2's last matmul — are not needed
    # until ~4 tiles worth of DR matmuls have been issued.
    def dr_mm(ps, jd, hn, jk):
        nc.tensor.matmul(
            ps[:],
            etil8[jk][:, :, jd * P:(jd + 1) * P],
            h8[jk][:, :, hn * 512:(hn + 1) * 512],
            start=(jk == 0),
            stop=(jk == 3),
            perf_mode=DR,
        )

    def dr_evict(ps, jd, hn):
        st = stage.tile([P, 512], FP16, tag="eost")
        nc.scalar.activation(st[:], ps[:], AF.Copy, bias=0.0, scale=B_AFF / KQ)
        nc.scalar.dma_start(out=eor[jd, :, hn * 512:(hn + 1) * 512], in_=st[:])

    head = [0, 1]
    head_ps = {}
    for jd in head:
        for hn in range(2):
            head_ps[(jd, hn)] = pmm.tile(
                [P, 512], F32, tag="mm", bufs=4, name=f"ps_h{jd}_{hn}"
            )
        for jk in range(3):
            for hn in range(2):
                dr_mm(head_ps[(jd, hn)], jd, hn, jk)
    for jd in head:
        for hn in range(2):
            dr_mm(head_ps[(jd, hn)], jd, hn, 3)
        for hn in range(2):
            dr_evict(head_ps[(jd, hn)], jd, hn)
    for jd in range(2, NT):
        psA = pmm.tile([P, 512], F32, tag="mm", bufs=4, name=f"ps_a{jd}")
        psB = pmm.tile([P, 512], F32, tag="mm", bufs=4, name=f"ps_b{jd}")
        for jk in range(4):
            dr_mm(psA, jd, 0, jk)
            dr_mm(psB, jd, 1, jk)
        dr_evict(psA, jd, 0)
        dr_evict(psB, jd, 1)


_NC_CACHE = None


def _get_nc():
    global _NC_CACHE
    if _NC_CACHE is None:
        _NC_CACHE = _build_nc()
    return _NC_CACHE


def kernel(z: np.ndarray, e: np.ndarray, M: np.ndarray):
    z = np.ascontiguousarray(np.asarray(z, dtype=np.float32))
    e = np.ascontiguousarray(np.asarray(e, dtype=np.float32))
    M = np.ascontiguousarray(np.asarray(M, dtype=np.float32))
    assert z.shape == (NC, SZ, SZ) and e.shape == (NC, SZ, SZ) and M.shape == (SZ, SZ)

    # host-side shard layout: fp16 shards, z and e also transposed.
    z16 = z.astype(np.float16)
    e16h = e.astype(np.float16)
    M16 = M.astype(np.float16)
    zT = np.ascontiguousarray(z16.transpose(0, 2, 1))
    eT = np.ascontiguousarray(e16h.transpose(0, 2, 1))

    nc = _get_nc()
    in_maps = [{"zT": zT[i], "e": e16h[i], "eT": eT[i], "M": M16}
               for i in range(NC)]
    res = run_bass_kernel_spmd(nc, in_maps, core_ids=list(range(NC))).results

    # device stores A and eo transposed ([m,n] / [d,n]); undo in the gather.
    A = np.stack([res[i]["A"] for i in range(NC)]).astype(np.float32)
    A = np.ascontiguousarray(A.transpose(0, 2, 1))
    eo = np.stack([res[i]["eoT"] for i in range(NC)]).astype(np.float32)
    eo = eo.transpose(0, 2, 1)
    # rank-1 c*colsum term: eo[n,d] += c * sum_m e[m,d]/S[m]  (exact, fp32)
    for i in range(NC):
        S_flat = res[i]["S"].astype(np.float64).T.reshape(-1)   # S[m], m=jm*128+p
        colsum = (1.0 / S_flat) @ e[i].astype(np.float64)
        eo[i] += (C_AFF * colsum)[None, :].astype(np.float32)
    return np.ascontiguousarray(eo), A


def _emit_body_v1(nc, big, pmm, tpool, stage, ztr, er, etr, mr, eor, ar, s_d,
                  e16, eT16, zMT, u16, aT16, h8, etil8,
                  halfb, zerob, S, S0, S1, r, zT16, m16):
    for j in range(NT):
        nc.sync.dma_start(out=zT16[:, j, :], in_=ztr[j])
        nc.sync.dma_start(out=m16[:, j, :], in_=mr[j])
    for j in range(NT):
        nc.sync.dma_start(out=eT16[:, j, :], in_=etr[j])
    for j in range(NT):
        nc.sync.dma_start(out=e16[:, j, :], in_=er[j])

    for h in range(2):
        for je in range(NT):
            ps = pmm.tile([P, 512], F32, tag="mm", bufs=4)
            for jd in range(NT):
                nc.tensor.matmul(
                    ps[:],
                    m16[:, jd, je * P:(je + 1) * P],
                    zT16[:, jd, h * 512:(h + 1) * 512],
                    start=(jd == 0),
                    stop=(jd == NT - 1),
                )
            nc.scalar.copy(out=zMT[:, je, h * 512:(h + 1) * 512], in_=ps[:])

    for jm in range(NT):
        for h in range(2):
            ps = pmm.tile([P, 512], F32, tag="mm", bufs=4)
            for je in range(NT):
                nc.tensor.matmul(
                    ps[:],
                    eT16[:, je, jm * P:(jm + 1) * P],
                    zMT[:, je, h * 512:(h + 1) * 512],
                    start=(je == 0),
                    stop=(je == NT - 1),
                )
            nc.scalar.activation(
                u16[:, jm, h * 512:(h + 1) * 512], ps[:], AF.Tanh,
                bias=zerob[:], scale=0.5,
            )
        t = tpool.tile([P, SZ], FP16, tag="t")
        nc.scalar.activation(
            t[:], u16[:, jm, :], AF.Exp,
            bias=halfb[:], scale=0.5,
            accum_out=S[:, jm:jm + 1],
        )
        nc.vector.reciprocal(r[:, jm:jm + 1], S[:, jm:jm + 1])
        nc.vector.tensor_scalar_mul(aT16[:, jm, :], t[:], r[:, jm:jm + 1])
        nc.sync.dma_start(out=ar[jm], in_=aT16[:, jm, :])
        nc.vector.tensor_scalar(
            h8[:, jm, :], t[:], C_AFF, 1.0 / B_AFF,
            op0=ALU.subtract, op1=ALU.mult,
        )
        nc.vector.tensor_scalar(
            etil8[:, jm, :], e16[:, jm, :], r[:, jm:jm + 1], KQ,
            op0=ALU.mult, op1=ALU.mult,
        )

    nc.sync.dma_start(out=s_d[:], in_=S[:])

    for jd in range(NT):
        for hn in range(2):
            ps = pmm.tile([P, 512], F32, tag="mm", bufs=4)
            for jk in range(4):
                nc.tensor.matmul(
                    ps[:],
                    etil8[:, 2 * jk:2 * jk + 2, jd * P:(jd + 1) * P],
                    h8[:, 2 * jk:2 * jk + 2, hn * 512:(hn + 1) * 512],
                    start=(jk == 0),
                    stop=(jk == 3),
                    perf_mode=DR,
                )
            st = stage.tile([P, 512], FP16, tag="eost")
            nc.vector.tensor_scalar_mul(st[:], ps[:], B_AFF / KQ)
            nc.sync.dma_start(out=eor[jd, :, hn * 512:(hn + 1) * 512], in_=st[:])


# revision 13
# speedup vs baseline: 1.1382x; 1.1382x over previous
"""Trainium2 Bass kernel for nn_Attention_4243427688485.

Computation (per batch b):
    a   = z_b @ M @ e_b^T            [N, ME]
    A   = softmax(sigmoid(a), dim=N) (softmax over the query axis N)
    eo  = A @ e_b                    [N, D]
Returns (eo, A) stacked over the batch.

Sharding: data-parallel over batch B=8 across the 8 NeuronCores (one batch
per core, M replicated).  No collectives.  Host uploads fp16 shards
(z/e pre-transposed); outputs come back fp16 and transposed where noted.

Per-core device program:
  - mm1 (fp16): zMT[e',n] = sum_d M[d,e'] z[n,d]
  - mm2 (fp16): aT[m,n]   = sum_e' e[m,e'] zM[n,e']; ScalarE evicts via
    tanh(a/2); softmax over n: t = exp(0.5u+0.5), accum_out row-sum S,
    DVE reciprocal r=1/S, aT16 = t*r (fp16) = the A output (transposed).
  - mm3 (fp8 DoubleRow, ~1.5-2x TensorE rate): exploits the sigmoid
    saturation structure.  t = exp(sigmoid(a)) clusters at exactly {1, e}
    (98.7% of entries saturate), so with the affine split
        t = c + beta*h,  c=(1+e)/2, beta=(e-1)/2,  h in {-1,+1} (mostly)
    h is EXACTLY representable in fp8e4 at the clusters.  Then
        eo[n,d] = c * colsum[d] + beta * sum_m h[m,n] * (e[m,d]/S[m])
    The beta-term runs as fp8e4 DoubleRow matmuls in the eoT orientation:
        eoT = (etil8)^T-style matmul: lhsT = etil8[m,d] = fp8(e*r*2048),
        rhs = h8[m,n], psum accumulates m in 4 double-row (256-wide) steps.
    The c*colsum term is rank-1 in n: the device outputs S ([128,8] fp32,
    4KB) and the host folds  eo += c * ((1/S) @ e)  exactly in fp32 during
    the gather (1M MACs/batch on host, negligible).
    fp8 quantization error on etil dominates: measured rel_err(eo) ~1.1e-2
    vs the 2e-2 gate (A output unchanged at ~2.8e-3).
"""

import numpy as np

import concourse.bass as bass
import concourse.mybir as mybir
import concourse.tile as tile
from concourse import bacc
from concourse.bass_utils import run_bass_kernel_spmd

AF = mybir.ActivationFunctionType
ALU = mybir.AluOpType
DR = mybir.MatmulPerfMode.DoubleRow
F32 = mybir.dt.float32
FP16 = mybir.dt.float16
FP8 = mybir.dt.float8e4

P = 128
NT = 8
SZ = 1024
NC = 8

C_AFF = (1.0 + float(np.e)) / 2.0     # 1.8591409142295225
B_AFF = (float(np.e) - 1.0) / 2.0     # 0.8591409142295225
KQ = 2048.0                           # etil prescale so fp8e4 sees ~unit values


def _build_nc(unroll: int = 1, tiny_io: bool = False, variant: str = "v2") -> bass.Bass:
    nc = bacc.Bacc()

    if tiny_io:
        nc.declare_dram_parameter("tin", [1, 1], F32, isOutput=False)
        dout = nc.declare_dram_parameter("tout", [1, 1], F32, isOutput=True)
        zt_d = nc.dram_tensor("zti", [SZ, SZ], FP16)
        e_d = nc.dram_tensor("ei", [SZ, SZ], FP16)
        et_d = nc.dram_tensor("eti", [SZ, SZ], FP16)
        m_d = nc.dram_tensor("Mi", [SZ, SZ], FP16)
        eo_d = nc.dram_tensor("eoi", [SZ, SZ], FP16)
        a_d = nc.dram_tensor("Ai", [SZ, SZ], FP16)
        s_d = nc.dram_tensor("Si", [P, NT], F32)
    else:
        zt_d = nc.declare_dram_parameter("zT", [SZ, SZ], FP16, isOutput=False)
        e_d = nc.declare_dram_parameter("e", [SZ, SZ], FP16, isOutput=False)
        et_d = nc.declare_dram_parameter("eT", [SZ, SZ], FP16, isOutput=False)
        m_d = nc.declare_dram_parameter("M", [SZ, SZ], FP16, isOutput=False)
        eo_d = nc.declare_dram_parameter("eoT", [SZ, SZ], FP16, isOutput=True)
        a_d = nc.declare_dram_parameter("A", [SZ, SZ], FP16, isOutput=True)
        s_d = nc.declare_dram_parameter("S", [P, NT], F32, isOutput=True)

    ztr = zt_d.rearrange("(j p) d -> j p d", p=P)
    er = e_d.rearrange("(j p) d -> j p d", p=P)
    etr = et_d.rearrange("(j p) d -> j p d", p=P)
    mr = m_d.rearrange("(j p) d -> j p d", p=P)
    eor = eo_d.rearrange("(j p) d -> j p d", p=P)
    ar = a_d.rearrange("(j p) d -> j p d", p=P)

    with tile.TileContext(nc) as tc:
        with (
            tc.tile_pool(name="big", bufs=1) as big,
            tc.tile_pool(name="consts", bufs=1) as consts,
            tc.tile_pool(name="tpool", bufs=4) as tpool,
            tc.tile_pool(name="stage", bufs=8) as stage,
            tc.tile_pool(name="psum_mm", bufs=1, space="PSUM") as pmm,
        ):
            halfb = consts.tile([P, 1], F32)
            nc.any.memset(halfb, 0.5)
            zerob = consts.tile([P, 1], F32)
            nc.any.memset(zerob, 0.0)
            S = consts.tile([P, NT], F32)
            S0 = consts.tile([P, NT], F32)
            S1 = consts.tile([P, NT], F32)
            r = consts.tile([P, NT], F32)

            # v2: zT16/m16 double-buffered (allocated per body in the body
            # emitter); v1 allocates them here as singles.
            if variant == "v1":
                zT16s = big.tile([P, NT, SZ], FP16, name="zT16s")
                m16s = big.tile([P, NT, SZ], FP16, name="m16s")
            else:
                zT16s = m16s = None
            e16 = big.tile([P, NT, SZ], FP16)    # e16[p, jm, d]  = e[jm*128+p, d]
            eT16 = big.tile([P, NT, SZ], FP16)   # eT16[p, je, m] = e[m, je*128+p]
            zMT = big.tile([P, NT, SZ], FP16)    # zMT[p, je, n]  = (z@M)[n, je*128+p]
            u16 = big.tile([P, NT, SZ], FP16)    # u[p, jm, n]    = tanh(a[n, jm*128+p]/2)
            aT16 = big.tile([P, NT, SZ], FP16)   # aT16[p, jm, n] = A[n, jm*128+p]
            # h8/etil8 as 4 per-pair tiles (matching DR jk pairs) so mm3's
            # early matmuls don't wait on the last m-tile's writes (tile
            # dependency tracking is per-tile, not per-slice).
            if variant == "v1":
                h8 = big.tile([P, NT, SZ], FP8, name="h8")
                etil8 = big.tile([P, NT, SZ], FP8, name="etil8")
            else:
                h8 = [big.tile([P, 2, SZ], FP8, name=f"h8_{i}") for i in range(4)]
                etil8 = [big.tile([P, 2, SZ], FP8, name=f"etil8_{i}") for i in range(4)]

            emit = _emit_body if variant == "v2" else _emit_body_v1
            for _ in range(unroll):
                emit(
                    nc, big, pmm, tpool, stage,
                    ztr, er, etr, mr, eor, ar, s_d,
                    e16, eT16, zMT, u16, aT16, h8, etil8,
                    halfb, zerob, S, S0, S1, r, zT16s, m16s,
                )

            if tiny_io:
                dstage = consts.tile([1, 1], F32)
                nc.any.memset(dstage, 1.0)
                nc.sync.dma_start(out=dout[:], in_=dstage[:])

    nc.compile()
    return nc


def _emit_body(nc, big, pmm, tpool, stage, ztr, er, etr, mr, eor, ar, s_d,
               e16, eT16, zMT, u16, aT16, h8, etil8,
               halfb, zerob, S, S0, S1, r, zT16s=None, m16s=None):
    zT16 = big.tile([P, NT, SZ], FP16, tag="zT16", bufs=2, name="zT16")
    m16 = big.tile([P, NT, SZ], FP16, tag="m16", bufs=2, name="m16")
    # ---- loads, in consumption order; zT on the SP DMA queue and m16 on
    # the Activation DMA queue so the mm1 head is fed by two queues ----
    for j in range(NT):
        nc.sync.dma_start(out=zT16[:, j, :], in_=ztr[j])
        nc.scalar.dma_start(out=m16[:, j, :], in_=mr[j])
    for j in range(NT):
        nc.sync.dma_start(out=eT16[:, j, :], in_=etr[j])
    for j in range(NT):
        nc.sync.dma_start(out=e16[:, j, :], in_=er[j])

    # ---- mm1: zMT[e', n] = sum_d M[d, e'] * z[n, d] ----
    for h in range(2):
        for je in range(NT):
            ps = pmm.tile([P, 512], F32, tag="mm", bufs=4)
            for jd in range(NT):
                nc.tensor.matmul(
                    ps[:],
                    m16[:, jd, je * P:(je + 1) * P],
                    zT16[:, jd, h * 512:(h + 1) * 512],
                    start=(jd == 0),
                    stop=(jd == NT - 1),
                )
            nc.scalar.copy(out=zMT[:, je, h * 512:(h + 1) * 512], in_=ps[:])

    # ---- mm2 + fused softmax(sigmoid) per m-tile ----
    for jm in range(NT):
        for h in range(2):
            ps = pmm.tile([P, 512], F32, tag="mm", bufs=4)
            for je in range(NT):
                nc.tensor.matmul(
                    ps[:],
                    eT16[:, je, jm * P:(jm + 1) * P],
                    zMT[:, je, h * 512:(h + 1) * 512],
                    start=(je == 0),
                    stop=(je == NT - 1),
                )
            nc.scalar.activation(
                u16[:, jm, h * 512:(h + 1) * 512], ps[:], AF.Tanh,
                bias=zerob[:], scale=0.5,
            )
        # exp + h8 in n-halves so DVE pipelines behind ScalarE, shortening
        # the jm=7 critical chain into mm3's jk=3 operands.
        t = tpool.tile([P, SZ], FP16, tag="t")
        for hh, Sh in ((0, S0), (1, S1)):
            nc.scalar.activation(
                t[:, hh * 512:(hh + 1) * 512],
                u16[:, jm, hh * 512:(hh + 1) * 512], AF.Exp,
                bias=halfb[:], scale=0.5,
                accum_out=Sh[:, jm:jm + 1],
            )
            nc.vector.tensor_scalar(
                h8[jm // 2][:, jm % 2, hh * 512:(hh + 1) * 512],
                t[:, hh * 512:(hh + 1) * 512], C_AFF, 1.0 / B_AFF,
                op0=ALU.subtract, op1=ALU.mult,
            )
        nc.vector.tensor_scalar_add(S[:, jm:jm + 1], S0[:, jm:jm + 1], S1[:, jm:jm + 1])
        nc.vector.reciprocal(r[:, jm:jm + 1], S[:, jm:jm + 1])
        nc.vector.tensor_scalar(
            etil8[jm // 2][:, jm % 2, :], e16[:, jm, :], r[:, jm:jm + 1], KQ,
            op0=ALU.mult, op1=ALU.mult,
        )
        nc.vector.tensor_scalar_mul(aT16[:, jm, :], t[:], r[:, jm:jm + 1])
        # A output: the fp16 softmax tile goes out directly (stored
        # transposed; host fixes layout and upcasts)
        nc.sync.dma_start(out=ar[jm], in_=aT16[:, jm, :])

    # S out (host folds the rank-1 c*colsum term during the gather)
    nc.sync.dma_start(out=s_d[:], in_=S[:])

    # ---- mm3 (fp8 DoubleRow): eoT[d, n] = beta/KQ * sum_m etil8*h8 ----
    # The first four psum tiles are staged (jk=0..2 for all four, then the
    # jk=3 round) so the jk=3 operands — h8/etil8 of the last two m-tiles,
    # which are only ready ~2us after mm2's last matmul — are not needed
    # until ~4 tiles worth of DR matmuls have been issued.
    def dr_mm(ps, jd, hn, jk):
        nc.tensor.matmul(
            ps[:],
            etil8[jk][:, :, jd * P:(jd + 1) * P],
            h8[jk][:, :, hn * 512:(hn + 1) * 512],
            start=(jk == 0),
            stop=(jk == 3),
            perf_mode=DR,
        )

    def dr_evict(ps, jd, hn):
        st = stage.tile([P, 512], FP16, tag="eost")
        nc.scalar.activation(st[:], ps[:], AF.Copy, bias=0.0, scale=B_AFF / KQ)
        nc.sync.dma_start(out=eor[jd, :, hn * 512:(hn + 1) * 512], in_=st[:])

    head = [0, 1]
    head_ps = {}
    for jd in head:
        for hn in range(2):
            head_ps[(jd, hn)] = pmm.tile(
                [P, 512], F32, tag="mm", bufs=4, name=f"ps_h{jd}_{hn}"
            )
        for jk in range(3):
            for hn in range(2):
                dr_mm(head_ps[(jd, hn)], jd, hn, jk)
    for jd in head:
        for hn in range(2):
            dr_mm(head_ps[(jd, hn)], jd, hn, 3)
        for hn in range(2):
            dr_evict(head_ps[(jd, hn)], jd, hn)
    for jd in range(2, NT):
        psA = pmm.tile([P, 512], F32, tag="mm", bufs=4, name=f"ps_a{jd}")
        psB = pmm.tile([P, 512], F32, tag="mm", bufs=4, name=f"ps_b{jd}")
        for jk in range(4):
            dr_mm(psA, jd, 0, jk)
            dr_mm(psB, jd, 1, jk)
        dr_evict(psA, jd, 0)
        dr_evict(psB, jd, 1)


_NC_CACHE = None


def _get_nc():
    global _NC_CACHE
    if _NC_CACHE is None:
        _NC_CACHE = _build_nc()
    return _NC_CACHE


def kernel(z: np.ndarray, e: np.ndarray, M: np.ndarray):
    z = np.ascontiguousarray(np.asarray(z, dtype=np.float32))
    e = np.ascontiguousarray(np.asarray(e, dtype=np.float32))
    M = np.ascontiguousarray(np.asarray(M, dtype=np.float32))
    assert z.shape == (NC, SZ, SZ) and e.shape == (NC, SZ, SZ) and M.shape == (SZ, SZ)

    # host-side shard layout: fp16 shards, z and e also transposed.
    z16 = z.astype(np.float16)
    e16h = e.astype(np.float16)
    M16 = M.astype(np.float16)
    zT = np.ascontiguousarray(z16.transpose(0, 2, 1))
    eT = np.ascontiguousarray(e16h.transpose(0, 2, 1))

    nc = _get_nc()
    in_maps = [{"zT": zT[i], "e": e16h[i], "eT": eT[i], "M": M16}
               for i in range(NC)]
    res = run_bass_kernel_spmd(nc, in_maps, core_ids=list(range(NC))).results

    # device stores A and eo transposed ([m,n] / [d,n]); undo in the gather.
    A = np.stack([res[i]["A"] for i in range(NC)]).astype(np.float32)
    A = np.ascontiguousarray(A.transpose(0, 2, 1))
    eo = np.stack([res[i]["eoT"] for i in range(NC)]).astype(np.float32)
    eo = eo.transpose(0, 2, 1)
    # rank-1 c*colsum term: eo[n,d] += c * sum_m e[m,d]/S[m]  (exact, fp32)
    for i in range(NC):
        S_flat = res[i]["S"].astype(np.float64).T.reshape(-1)   # S[m], m=jm*128+p
        colsum = (1.0 / S_flat) @ e[i].astype(np.float64)
        eo[i] += (C_AFF * colsum)[None, :].astype(np.float32)
    return np.ascontiguousarray(eo), A


def _emit_body_v1(nc, big, pmm, tpool, stage, ztr, er, etr, mr, eor, ar, s_d,
                  e16, eT16, zMT, u16, aT16, h8, etil8,
                  halfb, zerob, S, S0, S1, r, zT16, m16):
    for j in range(NT):
        nc.sync.dma_start(out=zT16[:, j, :], in_=ztr[j])
        nc.sync.dma_start(out=m16[:, j, :], in_=mr[j])
    for j in range(NT):
        nc.sync.dma_start(out=eT16[:, j, :], in_=etr[j])
    for j in range(NT):
        nc.sync.dma_start(out=e16[:, j, :], in_=er[j])

    for h in range(2):
        for je in range(NT):
            ps = pmm.tile([P, 512], F32, tag="mm", bufs=4)
            for jd in range(NT):
                nc.tensor.matmul(
                    ps[:],
                    m16[:, jd, je * P:(je + 1) * P],
                    zT16[:, jd, h * 512:(h + 1) * 512],
                    start=(jd == 0),
                    stop=(jd == NT - 1),
                )
            nc.scalar.copy(out=zMT[:, je, h * 512:(h + 1) * 512], in_=ps[:])

    for jm in range(NT):
        for h in range(2):
            ps = pmm.tile([P, 512], F32, tag="mm", bufs=4)
            for je in range(NT):
                nc.tensor.matmul(
                    ps[:],
                    eT16[:, je, jm * P:(jm + 1) * P],
                    zMT[:, je, h * 512:(h + 1) * 512],
                    start=(je == 0),
                    stop=(je == NT - 1),
                )
            nc.scalar.activation(
                u16[:, jm, h * 512:(h + 1) * 512], ps[:], AF.Tanh,
                bias=zerob[:], scale=0.5,
            )
        t = tpool.tile([P, SZ], FP16, tag="t")
        nc.scalar.activation(
            t[:], u16[:, jm, :], AF.Exp,
            bias=halfb[:], scale=0.5,
            accum_out=S[:, jm:jm + 1],
        )
        nc.vector.reciprocal(r[:, jm:jm + 1], S[:, jm:jm + 1])
        nc.vector.tensor_scalar_mul(aT16[:, jm, :], t[:], r[:, jm:jm + 1])
        nc.sync.dma_start(out=ar[jm], in_=aT16[:, jm, :])
        nc.vector.tensor_scalar(
            h8[:, jm, :], t[:], C_AFF, 1.0 / B_AFF,
            op0=ALU.subtract, op1=ALU.mult,
        )
        nc.vector.tensor_scalar(
            etil8[:, jm, :], e16[:, jm, :], r[:, jm:jm + 1], KQ,
            op0=ALU.mult, op1=ALU.mult,
        )

    nc.sync.dma_start(out=s_d[:], in_=S[:])

    for jd in range(NT):
        for hn in range(2):
            ps = pmm.tile([P, 512], F32, tag="mm", bufs=4)
            for jk in range(4):
                nc.tensor.matmul(
                    ps[:],
                    etil8[:, 2 * jk:2 * jk + 2, jd * P:(jd + 1) * P],
                    h8[:, 2 * jk:2 * jk + 2, hn * 512:(hn + 1) * 512],
                    start=(jk == 0),
                    stop=(jk == 3),
                    perf_mode=DR,
                )
            st = stage.tile([P, 512], FP16, tag="eost")
            nc.vector.tensor_scalar_mul(st[:], ps[:], B_AFF / KQ)
            nc.sync.dma_start(out=eor[jd, :, hn * 512:(hn + 1) * 512], in_=st[:])


# revision 14
# speedup vs baseline: 1.9851x; 1.7440x over previous
"""Trainium2 Bass kernel for nn_Attention_4243427688485.

Computation (per batch b):
    a   = z_b @ M @ e_b^T            [N, ME]
    A   = softmax(sigmoid(a), dim=N) (softmax over the query axis N)
    eo  = A @ e_b                    [N, D]
Returns (eo, A) stacked over the batch.

Sharding: data-parallel over batch B=8 across the 8 NeuronCores (one batch
per core, M replicated).  No collectives.  Host uploads fp16 shards
(z/e pre-transposed); outputs come back fp16 and transposed where noted.

Per-core device program:
  - mm1 (fp16): zMT[e',n] = sum_d M[d,e'] z[n,d]
  - mm2 (fp16): aT[m,n]   = sum_e' e[m,e'] zM[n,e']; ScalarE evicts via
    tanh(a/2); softmax over n: t = exp(0.5u+0.5), accum_out row-sum S,
    DVE reciprocal r=1/S, aT16 = t*r (fp16) = the A output (transposed).
  - mm3 (fp8 DoubleRow, ~1.5-2x TensorE rate): exploits the sigmoid
    saturation structure.  t = exp(sigmoid(a)) clusters at exactly {1, e}
    (98.7% of entries saturate), so with the affine split
        t = c + beta*h,  c=(1+e)/2, beta=(e-1)/2,  h in {-1,+1} (mostly)
    h is EXACTLY representable in fp8e4 at the clusters.  Then
        eo[n,d] = c * colsum[d] + beta * sum_m h[m,n] * (e[m,d]/S[m])
    The beta-term runs as fp8e4 DoubleRow matmuls in the eoT orientation:
        eoT = (etil8)^T-style matmul: lhsT = etil8[m,d] = fp8(e*r*2048),
        rhs = h8[m,n], psum accumulates m in 4 double-row (256-wide) steps.
    The c*colsum term is rank-1 in n: the device outputs S ([128,8] fp32,
    4KB) and the host folds  eo += c * ((1/S) @ e)  exactly in fp32 during
    the gather (1M MACs/batch on host, negligible).
    fp8 quantization error on etil dominates: measured rel_err(eo) ~1.1e-2
    vs the 2e-2 gate (A output unchanged at ~2.8e-3).

Scheduling (variant "v2", the default):
  - h8/etil8 live in 4 per-pair tiles matching the DoubleRow k-pairs, and
    mm3's first two jd-groups are emitted staged (jk=0..2 rounds first),
    so mm3 starts right at mm2's end instead of waiting ~2us for the last
    m-tile's fp8 operands (tile deps are per-tile, not per-slice).
  - exp/h8 run in n-halves so DVE pipelines behind ScalarE on the
    critical jm=7 chain; S = S0+S1.
  - zT16/m16 are double-buffered across unrolled bodies; m16 loads ride
    the Activation DMA queue so the mm1 head is fed by two queues.
  - mm3 psum eviction on ScalarE (AF.Copy, same table phase as the zMT
    copies), A/eoT/S output DMAs on the SP queue.
  Cost model (CoreSim): ~69.2us/body steady-state, ~75.9us single-exec
  vs ~84/~94.9us for the all-fp16 baseline.  "v1" (first fp8 revision,
  measured 84756ns vs baseline 98216ns on HW) is kept for A/B.
"""

import numpy as np

import concourse.bass as bass
import concourse.mybir as mybir
import concourse.tile as tile
from concourse import bacc
from concourse.bass_utils import run_bass_kernel_spmd

AF = mybir.ActivationFunctionType
ALU = mybir.AluOpType
DR = mybir.MatmulPerfMode.DoubleRow
F32 = mybir.dt.float32
FP16 = mybir.dt.float16
FP8 = mybir.dt.float8e4

P = 128
NT = 8
SZ = 1024
NC = 8

C_AFF = (1.0 + float(np.e)) / 2.0     # 1.8591409142295225
B_AFF = (float(np.e) - 1.0) / 2.0     # 0.8591409142295225
KQ = 2048.0                           # etil prescale so fp8e4 sees ~unit values


def _build_nc(unroll: int = 1, tiny_io: bool = False, variant: str = "v2") -> bass.Bass:
    nc = bacc.Bacc()

    if tiny_io:
        nc.declare_dram_parameter("tin", [1, 1], F32, isOutput=False)
        dout = nc.declare_dram_parameter("tout", [1, 1], F32, isOutput=True)
        zt_d = nc.dram_tensor("zti", [SZ, SZ], FP16)
        e_d = nc.dram_tensor("ei", [SZ, SZ], FP16)
        et_d = nc.dram_tensor("eti", [SZ, SZ], FP16)
        m_d = nc.dram_tensor("Mi", [SZ, SZ], FP16)
        eo_d = nc.dram_tensor("eoi", [SZ, SZ], FP16)
        a_d = nc.dram_tensor("Ai", [SZ, SZ], FP16)
        s_d = nc.dram_tensor("Si", [P, NT], F32)
    else:
        zt_d = nc.declare_dram_parameter("zT", [SZ, SZ], FP16, isOutput=False)
        e_d = nc.declare_dram_parameter("e", [SZ, SZ], FP16, isOutput=False)
        et_d = nc.declare_dram_parameter("eT", [SZ, SZ], FP16, isOutput=False)
        m_d = nc.declare_dram_parameter("M", [SZ, SZ], FP16, isOutput=False)
        eo_d = nc.declare_dram_parameter("eoT", [SZ, SZ], FP16, isOutput=True)
        a_d = nc.declare_dram_parameter("A", [SZ, SZ], FP16, isOutput=True)
        s_d = nc.declare_dram_parameter("S", [P, NT], F32, isOutput=True)

    ztr = zt_d.rearrange("(j p) d -> j p d", p=P)
    er = e_d.rearrange("(j p) d -> j p d", p=P)
    etr = et_d.rearrange("(j p) d -> j p d", p=P)
    mr = m_d.rearrange("(j p) d -> j p d", p=P)
    eor = eo_d.rearrange("(j p) d -> j p d", p=P)
    ar = a_d.rearrange("(j p) d -> j p d", p=P)

    with tile.TileContext(nc) as tc:
        with (
            tc.tile_pool(name="big", bufs=1) as big,
            tc.tile_pool(name="consts", bufs=1) as consts,
            tc.tile_pool(name="tpool", bufs=4) as tpool,
            tc.tile_pool(name="stage", bufs=8) as stage,
            tc.tile_pool(name="psum_mm", bufs=1, space="PSUM") as pmm,
        ):
            halfb = consts.tile([P, 1], F32)
            nc.any.memset(halfb, 0.5)
            zerob = consts.tile([P, 1], F32)
            nc.any.memset(zerob, 0.0)
            S = consts.tile([P, NT], F32)
            S0 = consts.tile([P, NT], F32)
            S1 = consts.tile([P, NT], F32)
            r = consts.tile([P, NT], F32)

            # v2: zT16/m16 double-buffered (allocated per body in the body
            # emitter); v1 allocates them here as singles.
            if variant == "v1":
                zT16s = big.tile([P, NT, SZ], FP16, name="zT16s")
                m16s = big.tile([P, NT, SZ], FP16, name="m16s")
            else:
                zT16s = m16s = None
            e16 = big.tile([P, NT, SZ], FP16)    # e16[p, jm, d]  = e[jm*128+p, d]
            eT16 = big.tile([P, NT, SZ], FP16)   # eT16[p, je, m] = e[m, je*128+p]
            zMT = big.tile([P, NT, SZ], FP16)    # zMT[p, je, n]  = (z@M)[n, je*128+p]
            u16 = big.tile([P, NT, SZ], FP16)    # u[p, jm, n]    = tanh(a[n, jm*128+p]/2)
            aT16 = big.tile([P, NT, SZ], FP16)   # aT16[p, jm, n] = A[n, jm*128+p]
            # h8/etil8 as 4 per-pair tiles (matching DR jk pairs) so mm3's
            # early matmuls don't wait on the last m-tile's writes (tile
            # dependency tracking is per-tile, not per-slice).
            if variant == "v1":
                h8 = big.tile([P, NT, SZ], FP8, name="h8")
                etil8 = big.tile([P, NT, SZ], FP8, name="etil8")
            else:
                h8 = [big.tile([P, 2, SZ], FP8, name=f"h8_{i}") for i in range(4)]
                etil8 = [big.tile([P, 2, SZ], FP8, name=f"etil8_{i}") for i in range(4)]

            emit = _emit_body if variant == "v2" else _emit_body_v1
            for _ in range(unroll):
                emit(
                    nc, big, pmm, tpool, stage,
                    ztr, er, etr, mr, eor, ar, s_d,
                    e16, eT16, zMT, u16, aT16, h8, etil8,
                    halfb, zerob, S, S0, S1, r, zT16s, m16s,
                )

            if tiny_io:
                dstage = consts.tile([1, 1], F32)
                nc.any.memset(dstage, 1.0)
                nc.sync.dma_start(out=dout[:], in_=dstage[:])

    nc.compile()
    return nc


def _emit_body(nc, big, pmm, tpool, stage, ztr, er, etr, mr, eor, ar, s_d,
               e16, eT16, zMT, u16, aT16, h8, etil8,
               halfb, zerob, S, S0, S1, r, zT16s=None, m16s=None):
    zT16 = big.tile([P, NT, SZ], FP16, tag="zT16", bufs=2, name="zT16")
    m16 = big.tile([P, NT, SZ], FP16, tag="m16", bufs=2, name="m16")
    # ---- loads, in consumption order; zT on the SP DMA queue and m16 on
    # the Activation DMA queue so the mm1 head is fed by two queues ----
    for j in range(NT):
        nc.sync.dma_start(out=zT16[:, j, :], in_=ztr[j])
        nc.scalar.dma_start(out=m16[:, j, :], in_=mr[j])
    for j in range(NT):
        nc.sync.dma_start(out=eT16[:, j, :], in_=etr[j])
    for j in range(NT):
        nc.sync.dma_start(out=e16[:, j, :], in_=er[j])

    # ---- mm1: zMT[e', n] = sum_d M[d, e'] * z[n, d] ----
    for h in range(2):
        for je in range(NT):
            ps = pmm.tile([P, 512], F32, tag="mm", bufs=4)
            for jd in range(NT):
                nc.tensor.matmul(
                    ps[:],
                    m16[:, jd, je * P:(je + 1) * P],
                    zT16[:, jd, h * 512:(h + 1) * 512],
                    start=(jd == 0),
                    stop=(jd == NT - 1),
                )
            nc.scalar.copy(out=zMT[:, je, h * 512:(h + 1) * 512], in_=ps[:])

    # ---- mm2 + fused softmax(sigmoid) per m-tile ----
    for jm in range(NT):
        for h in range(2):
            ps = pmm.tile([P, 512], F32, tag="mm", bufs=4)
            for je in range(NT):
                nc.tensor.matmul(
                    ps[:],
                    eT16[:, je, jm * P:(jm + 1) * P],
                    zMT[:, je, h * 512:(h + 1) * 512],
                    start=(je == 0),
                    stop=(je == NT - 1),
                )
            nc.scalar.activation(
                u16[:, jm, h * 512:(h + 1) * 512], ps[:], AF.Tanh,
                bias=zerob[:], scale=0.5,
            )
        # exp + h8 in n-halves so DVE pipelines behind ScalarE, shortening
        # the jm=7 critical chain into mm3's jk=3 operands.
        t = tpool.tile([P, SZ], FP16, tag="t")
        for hh, Sh in ((0, S0), (1, S1)):
            nc.scalar.activation(
                t[:, hh * 512:(hh + 1) * 512],
                u16[:, jm, hh * 512:(hh + 1) * 512], AF.Exp,
                bias=halfb[:], scale=0.5,
                accum_out=Sh[:, jm:jm + 1],
            )
            nc.vector.tensor_scalar(
                h8[jm // 2][:, jm % 2, hh * 512:(hh + 1) * 512],
                t[:, hh * 512:(hh + 1) * 512], C_AFF, 1.0 / B_AFF,
                op0=ALU.subtract, op1=ALU.mult,
            )
        nc.vector.tensor_scalar_add(S[:, jm:jm + 1], S0[:, jm:jm + 1], S1[:, jm:jm + 1])
        nc.vector.reciprocal(r[:, jm:jm + 1], S[:, jm:jm + 1])
        nc.vector.tensor_scalar(
            etil8[jm // 2][:, jm % 2, :], e16[:, jm, :], r[:, jm:jm + 1], KQ,
            op0=ALU.mult, op1=ALU.mult,
        )
        nc.vector.tensor_scalar_mul(aT16[:, jm, :], t[:], r[:, jm:jm + 1])
        # A output: the fp16 softmax tile goes out directly (stored
        # transposed; host fixes layout and upcasts)
        nc.sync.dma_start(out=ar[jm], in_=aT16[:, jm, :])

    # S out (host folds the rank-1 c*colsum term during the gather)
    nc.sync.dma_start(out=s_d[:], in_=S[:])

    # ---- mm3 (fp8 DoubleRow): eoT[d, n] = beta/KQ * sum_m etil8*h8 ----
    # The first four psum tiles are staged (jk=0..2 for all four, then the
    # jk=3 round) so the jk=3 operands — h8/etil8 of the last two m-tiles,
    # which are only ready ~2us after mm2's last matmul — are not needed
    # until ~4 tiles worth of DR matmuls have been issued.
    def dr_mm(ps, jd, hn, jk):
        nc.tensor.matmul(
            ps[:],
            etil8[jk][:, :, jd * P:(jd + 1) * P],
            h8[jk][:, :, hn * 512:(hn + 1) * 512],
            start=(jk == 0),
            stop=(jk == 3),
            perf_mode=DR,
        )

    def dr_evict(ps, jd, hn):
        st = stage.tile([P, 512], FP16, tag="eost")
        nc.scalar.activation(st[:], ps[:], AF.Copy, bias=0.0, scale=B_AFF / KQ)
        nc.sync.dma_start(out=eor[jd, :, hn * 512:(hn + 1) * 512], in_=st[:])

    head = [0, 1]
    head_ps = {}
    for jd in head:
        for hn in range(2):
            head_ps[(jd, hn)] = pmm.tile(
                [P, 512], F32, tag="mm", bufs=4, name=f"ps_h{jd}_{hn}"
            )
        for jk in range(3):
            for hn in range(2):
                dr_mm(head_ps[(jd, hn)], jd, hn, jk)
    for jd in head:
        for hn in range(2):
            dr_mm(head_ps[(jd, hn)], jd, hn, 3)
        for hn in range(2):
            dr_evict(head_ps[(jd, hn)], jd, hn)
    for jd in range(2, NT):
        psA = pmm.tile([P, 512], F32, tag="mm", bufs=4, name=f"ps_a{jd}")
        psB = pmm.tile([P, 512], F32, tag="mm", bufs=4, name=f"ps_b{jd}")
        for jk in range(4):
            dr_mm(psA, jd, 0, jk)
            dr_mm(psB, jd, 1, jk)
        dr_evict(psA, jd, 0)
        dr_evict(psB, jd, 1)


_NC_CACHE = None


def _get_nc():
    global _NC_CACHE
    if _NC_CACHE is None:
        _NC_CACHE = _build_nc()
    return _NC_CACHE


def kernel(z: np.ndarray, e: np.ndarray, M: np.ndarray):
    z = np.ascontiguousarray(np.asarray(z, dtype=np.float32))
    e = np.ascontiguousarray(np.asarray(e, dtype=np.float32))
    M = np.ascontiguousarray(np.asarray(M, dtype=np.float32))
    assert z.shape == (NC, SZ, SZ) and e.shape == (NC, SZ, SZ) and M.shape == (SZ, SZ)

    # host-side shard layout: fp16 shards, z and e also transposed.
    z16 = z.astype(np.float16)
    e16h = e.astype(np.float16)
    M16 = M.astype(np.float16)
    zT = np.ascontiguousarray(z16.transpose(0, 2, 1))
    eT = np.ascontiguousarray(e16h.transpose(0, 2, 1))

    nc = _get_nc()
    in_maps = [{"zT": zT[i], "e": e16h[i], "eT": eT[i], "M": M16}
               for i in range(NC)]
    res = run_bass_kernel_spmd(nc, in_maps, core_ids=list(range(NC))).results

    # device stores A and eo transposed ([m,n] / [d,n]); undo in the gather.
    A = np.stack([res[i]["A"] for i in range(NC)]).astype(np.float32)
    A = np.ascontiguousarray(A.transpose(0, 2, 1))
    eo = np.stack([res[i]["eoT"] for i in range(NC)]).astype(np.float32)
    eo = eo.transpose(0, 2, 1)
    # rank-1 c*colsum term: eo[n,d] += c * sum_m e[m,d]/S[m]  (exact, fp32)
    for i in range(NC):
        S_flat = res[i]["S"].astype(np.float64).T.reshape(-1)   # S[m], m=jm*128+p
        colsum = (1.0 / S_flat) @ e[i].astype(np.float64)
        eo[i] += (C_AFF * colsum)[None, :].astype(np.float32)
    return np.ascontiguousarray(eo), A


def _emit_body_v1(nc, big, pmm, tpool, stage, ztr, er, etr, mr, eor, ar, s_d,
                  e16, eT16, zMT, u16, aT16, h8, etil8,
                  halfb, zerob, S, S0, S1, r, zT16, m16):
    for j in range(NT):
        nc.sync.dma_start(out=zT16[:, j, :], in_=ztr[j])
        nc.sync.dma_start(out=m16[:, j, :], in_=mr[j])
    for j in range(NT):
        nc.sync.dma_start(out=eT16[:, j, :], in_=etr[j])
    for j in range(NT):
        nc.sync.dma_start(out=e16[:, j, :], in_=er[j])

    for h in range(2):
        for je in range(NT):
            ps = pmm.tile([P, 512], F32, tag="mm", bufs=4)
            for jd in range(NT):
                nc.tensor.matmul(
                    ps[:],
                    m16[:, jd, je * P:(je + 1) * P],
                    zT16[:, jd, h * 512:(h + 1) * 512],
                    start=(jd == 0),
                    stop=(jd == NT - 1),
                )
            nc.scalar.copy(out=zMT[:, je, h * 512:(h + 1) * 512], in_=ps[:])

    for jm in range(NT):
        for h in range(2):
            ps = pmm.tile([P, 512], F32, tag="mm", bufs=4)
            for je in range(NT):
                nc.tensor.matmul(
                    ps[:],
                    eT16[:, je, jm * P:(jm + 1) * P],
                    zMT[:, je, h * 512:(h + 1) * 512],
                    start=(je == 0),
                    stop=(je == NT - 1),
                )
            nc.scalar.activation(
                u16[:, jm, h * 512:(h + 1) * 512], ps[:], AF.Tanh,
                bias=zerob[:], scale=0.5,
            )
        t = tpool.tile([P, SZ], FP16, tag="t")
        nc.scalar.activation(
            t[:], u16[:, jm, :], AF.Exp,
            bias=halfb[:], scale=0.5,
            accum_out=S[:, jm:jm + 1],
        )
        nc.vector.reciprocal(r[:, jm:jm + 1], S[:, jm:jm + 1])
        nc.vector.tensor_scalar_mul(aT16[:, jm, :], t[:], r[:, jm:jm + 1])
        nc.sync.dma_start(out=ar[jm], in_=aT16[:, jm, :])
        nc.vector.tensor_scalar(
            h8[:, jm, :], t[:], C_AFF, 1.0 / B_AFF,
            op0=ALU.subtract, op1=ALU.mult,
        )
        nc.vector.tensor_scalar(
            etil8[:, jm, :], e16[:, jm, :], r[:, jm:jm + 1], KQ,
            op0=ALU.mult, op1=ALU.mult,
        )

    nc.sync.dma_start(out=s_d[:], in_=S[:])

    for jd in range(NT):
        for hn in range(2):
            ps = pmm.tile([P, 512], F32, tag="mm", bufs=4)
            for jk in range(4):
                nc.tensor.matmul(
                    ps[:],
                    etil8[:, 2 * jk:2 * jk + 2, jd * P:(jd + 1) * P],
                    h8[:, 2 * jk:2 * jk + 2, hn * 512:(hn + 1) * 512],
                    start=(jk == 0),
                    stop=(jk == 3),
                    perf_mode=DR,
                )
            st = stage.tile([P, 512], FP16, tag="eost")
            nc.vector.tensor_scalar_mul(st[:], ps[:], B_AFF / KQ)
            nc.sync.dma_start(out=eor[jd, :, hn * 512:(hn + 1) * 512], in_=st[:])
